# revision 5
# baseline (speedup 1.0000x reference)
"""MoE routing kernel for Trainium2 (8 NeuronCores, zero-collective design).

Reference computes (identity activation!):
    logits = x @ wg ; top-2 softmax gating
    h = x @ w1[e] + b1[e]; o = h @ w2[e] + b2[e]          (dense over experts)
    y = sum_e combine[n,e] * o[n,e,:] ; s = sum_d y ; out = log_softmax(s, T)

Because the final reduction over d is linear and the FFN has no nonlinearity,
    sum_d o[n,e,d] = x[n] . v[e] + c[e]
with v[e] = w1[e] @ w2s[e],  w2s[e] = sum_d w2[e,:,d],
     c[e] = b1[e] . w2s[e] + sum_d b2[e,d].
Gating: combine weights of the top-2 logits l0 >= l1 reduce to
     w0 = sigmoid(l0 - l1), w1 = 1 - w0   (softmax normalizers cancel).

Device plan: on this runtime each collective_compute costs ~1.2 ms (fixed
floor, size-independent), while streaming all expert weights costs only
~0.3 ms at DMA rate. So we use ZERO collectives: every core streams the
FULL w1/w2 (128 MB) and computes v/c for all 8 experts locally, then does
gating + log-softmax for one full batch row (cores 2b and 2b+1 both
compute row b; kernel() reads even cores and uses the odd twins as a
free bitwise consistency check).

Per-core pipeline (engine assignment chosen so everything overlaps the
weight stream, which is the critical path):
  - x row tiles DMA (gpsimd ring) -> PE transposes -> xT resident in SBUF.
  - per expert e: w2[e] tiles (scalar/ACT HWDGE ring) -> ACT Copy+accum
    rowsums -> w2s[e]; bounce+broadcast (gpsimd); w1[e] tiles (sync HWDGE
    ring) -> DVE tensor_tensor_reduce against broadcast w2s -> v[e].
  - one fused matmul per token tile against [wg | v] -> logits + expert
    sums, top-2 gating on DVE/ACT, log-softmax over the row, write y.
"""

import numpy as np

import concourse.bass as bass
import concourse.tile as tile
from concourse import mybir
from concourse.bass_utils import run_bass_kernel_spmd
from concourse.masks import make_identity

B, T, D, H, E = 4, 1024, 1024, 2048, 8
N = B * T
NCORES = 8
NTOK = T  # each core computes one full batch row
F32 = mybir.dt.float32
AX = mybir.AxisListType
OP = mybir.AluOpType
ACTF = mybir.ActivationFunctionType

# tensor_tensor_reduce lowers to a raw-ISA instruction that this walrus
# build cannot encode ("ISA wrong length" in visitInstISA) -- use the
# two-pass mul+reduce everywhere instead.
USE_TTR = False
ND = D // 128   # 8 d-chunks
NH = H // 128   # 16 h-chunks
NT = T // 128   # 8 token tiles
JW1 = 2         # d-chunks per w1 DMA tile (1 MB)
JW2 = 4         # h-chunks per w2 DMA tile (2 MB)

_CACHE = {}


def _legalize_waits(nc):
    """Walrus accepts only one sync-wait slot on most TRN2 instruction
    encodings. Move surplus waits onto an InstDrain inserted immediately
    before the offender on the same engine (drains accept many waits -- the
    Tile tail barrier relies on that). Same-engine order is preserved, so
    semantics are unchanged."""
    # EVENT_SEMAPHORE_RANGE_CLEAR (isa opcode 176) crashes this runtime
    # (NRT_EXEC_UNIT_UNRECOVERABLE); the is_reset_sema drain already resets
    # the tile sems, and the barrier butterfly leaves its event sems at 0,
    # so dropping it is safe (verified over repeated executions).
    for bb in nc.main_func.blocks:
        bb.instructions = [i for i in bb.instructions
                           if "EVENT_SEMAPHORE_RANGE_CLEAR" not in str(i)]
    offenders = []
    for bb in nc.main_func.blocks:
        for inst in bb.instructions:
            si = inst.sync_info
            if si is None:
                continue
            if len(si.on_wait) > 1:
                offenders.append((bb, inst))
    import bass_rust as _br
    for bb, inst in offenders:
        si = inst.sync_info
        waits = list(si.on_wait)
        si.on_wait = [waits[-1]]
        idx = bb.instructions.index(inst)
        for w in reversed(waits[:-1]):
            d = nc.engines[inst.engine].nop(nofuse=True, hint="wait_legalize")
            dins = d.ins
            for bb2 in nc.main_func.blocks:
                if dins in bb2.instructions:
                    bb2.instructions.remove(dins)
            dins.sync_info = _br.SyncInfo(on_wait=[w], on_update=[])
            bb.instructions.insert(idx, dins)


def _build_nc(reps: int = 1, variant: str = "full") -> bass.Bass:
    nc = bass.Bass("TRN2", target_bir_lowering=False)

    xrow = nc.dram_tensor("xrow", [T, D], F32, kind="ExternalInput")
    wg = nc.dram_tensor("wg", [D, E], F32, kind="ExternalInput")
    w1f = nc.dram_tensor("w1f", [E * D, H], F32, kind="ExternalInput")
    w2f = nc.dram_tensor("w2f", [E * H, D], F32, kind="ExternalInput")
    b1f = nc.dram_tensor("b1f", [E, H], F32, kind="ExternalInput")
    b2f = nc.dram_tensor("b2f", [E, D], F32, kind="ExternalInput")
    yout = nc.dram_tensor("y", [T], F32, kind="ExternalOutput")

    with tile.TileContext(nc) as tc:
      for _rep in range(reps):
        w2s_dram = nc.dram_tensor(f"w2s_dram_{_rep}", [E, H], F32)
        c_dram = nc.dram_tensor(f"c_dram_{_rep}", [1, E], F32)
        s_dram = nc.dram_tensor(f"s_dram_{_rep}", [T], F32)
        with (
            tc.tile_pool(name="singles", bufs=1) as singles,
            tc.tile_pool(name="w2pool", bufs=2) as w2pool,
            tc.tile_pool(name="w1pool", bufs=3) as w1pool,
            tc.tile_pool(name="wspool", bufs=2) as wspool,
            tc.tile_pool(name="w2cpool", bufs=2) as w2cpool,
            tc.tile_pool(name="vpool", bufs=2) as vpool,
            tc.tile_pool(name="xpool", bufs=2) as xpool,
            tc.tile_pool(name="gpool", bufs=2) as gpool,
            tc.tile_pool(name="spool", bufs=2) as spool,
            tc.tile_pool(name="lpool", bufs=1) as lpool,
            tc.tile_pool(name="psT", bufs=2, space="PSUM") as psT,
            tc.tile_pool(name="psO", bufs=2, space="PSUM") as psO,
        ):
            ident = singles.tile([128, 128], F32)
            make_identity(nc, ident)

            # ---- token tiles: DMA + PE transpose, fully overlapped with
            # the (later, much longer) weight stream ----
            xT = singles.tile([128, NT, ND, 128], F32)
            for jn in range(NT):
                xt = xpool.tile([128, D], F32)
                nc.gpsimd.dma_start(out=xt, in_=xrow[jn * 128:(jn + 1) * 128, :])
                if variant == "dma":
                    continue
                for jd in range(ND):
                    pst = psT.tile([128, 128], F32)
                    nc.tensor.transpose(pst, xt[:, jd * 128:(jd + 1) * 128], ident)
                    nc.vector.tensor_copy(out=xT[:, jn, jd, :], in_=pst)

            # lhsT [128, ND, 2E]: columns [wg | v]
            wsb = singles.tile([128, ND, 2 * E], F32)
            if variant != "dma":
                nc.gpsimd.dma_start(
                    out=wsb[:, :, 0:E],
                    in_=wg.ap().rearrange("(j p) e -> p j e", p=128),
                )

            # ---- Phase A: per-expert v (all 8 experts on every core) ----
            for e in range(E):
                w2s_cols = w2cpool.tile([128, NH], F32)
                for t in range(0, NH, JW2):
                    w2t = w2pool.tile([128, JW2, D], F32)
                    nc.scalar.dma_start(
                        out=w2t,
                        in_=w2f[e * H + t * 128: e * H + (t + JW2) * 128, :]
                            .rearrange("(j p) d -> p j d", p=128),
                    )
                    if variant == "dma":
                        continue
                    for jj in range(JW2):
                        nc.scalar.activation(
                            out=w2t[:, jj, :], in_=w2t[:, jj, :], func=ACTF.Copy,
                            accum_out=w2s_cols[:, t + jj: t + jj + 1],
                        )
                if variant != "dma":
                    nc.gpsimd.dma_start(
                        out=w2s_dram[e:e + 1, :]
                            .rearrange("one (j p) -> p (one j)", p=128),
                        in_=w2s_cols,
                    )
                    w2s_b = wspool.tile([128, H], F32)
                    nc.gpsimd.dma_start(
                        out=w2s_b, in_=w2s_dram[e:e + 1, :].to_broadcast((128, H))
                    )
                    v_cols = vpool.tile([128, ND], F32)
                for t in range(0, ND, JW1):
                    w1t = w1pool.tile([128, JW1, H], F32)
                    nc.sync.dma_start(
                        out=w1t,
                        in_=w1f[e * D + t * 128: e * D + (t + JW1) * 128, :]
                            .rearrange("(j p) h -> p j h", p=128),
                    )
                    if variant == "dma":
                        continue
                    for jj in range(JW1):
                        if USE_TTR:
                            nc.vector.tensor_tensor_reduce(
                                out=w1t[:, jj, :], in0=w1t[:, jj, :], in1=w2s_b,
                                scale=1.0, scalar=0.0, op0=OP.mult, op1=OP.add,
                                accum_out=v_cols[:, t + jj: t + jj + 1],
                            )
                        else:
                            nc.vector.tensor_mul(
                                out=w1t[:, jj, :], in0=w1t[:, jj, :], in1=w2s_b)
                            nc.vector.tensor_reduce(
                                out=v_cols[:, t + jj: t + jj + 1],
                                in_=w1t[:, jj, :], axis=AX.X, op=OP.add)
                if variant != "dma":
                    nc.vector.tensor_copy(out=wsb[:, :, E + e], in_=v_cols)

            if variant == "dma":
                ydummy = lpool.tile([1, T], F32)
                nc.vector.memset(ydummy, 0.0)
                nc.gpsimd.dma_start(out=yout.ap(), in_=ydummy)
                continue

            # ---- c[e] = b1[e] . w2s[e] + sum(b2[e]) ----
            w2s_all = lpool.tile([E, H], F32)
            nc.gpsimd.dma_start(out=w2s_all, in_=w2s_dram.ap())
            b1sb = lpool.tile([E, H], F32)
            nc.sync.dma_start(out=b1sb, in_=b1f.ap())
            b2sb = lpool.tile([E, D], F32)
            nc.sync.dma_start(out=b2sb, in_=b2f.ap())
            c1 = lpool.tile([E, 1], F32)
            nc.vector.tensor_mul(out=b1sb, in0=b1sb, in1=w2s_all)
            nc.vector.tensor_reduce(out=c1, in_=b1sb, axis=AX.X, op=OP.add)
            c2 = lpool.tile([E, 1], F32)
            nc.vector.tensor_reduce(out=c2, in_=b2sb, axis=AX.X, op=OP.add)
            csum = lpool.tile([E, 1], F32)
            nc.vector.tensor_add(out=csum, in0=c1, in1=c2)
            nc.gpsimd.dma_start(
                out=c_dram.ap().rearrange("one e -> e one"), in_=csum)
            c_b = singles.tile([128, E], F32)
            nc.gpsimd.dma_start(out=c_b, in_=c_dram.ap().to_broadcast((128, E)))

            if variant == "phaseA":
                ydummy = lpool.tile([1, T], F32)
                nc.vector.memset(ydummy, 0.0)
                nc.gpsimd.dma_start(out=yout.ap(), in_=ydummy)
                continue

            # ---- Phase B: gating for all NT token tiles ----
            s_cols = singles.tile([128, NT], F32)
            for jn in range(NT):
                pso = psO.tile([2 * E, 128], F32)
                for jd in range(ND):
                    nc.tensor.matmul(
                        pso, lhsT=wsb[:, jd, :], rhs=xT[:, jn, jd, :],
                        start=(jd == 0), stop=(jd == ND - 1),
                    )
                # transpose [16, n] -> [n, 16] for per-token gating
                gi = gpool.tile([2 * E, 128], F32)
                nc.scalar.copy(out=gi, in_=pso)
                psg = psT.tile([128, 2 * E], F32)
                nc.tensor.transpose(psg, gi, ident[0:2 * E, 0:2 * E])
                Ls = psg[:, 0:E]
                S2 = gpool.tile([128, E], F32)
                nc.vector.tensor_add(out=S2, in0=psg[:, E:2 * E], in1=c_b)

                m0 = spool.tile([128, 1], F32)
                nc.vector.tensor_reduce(out=m0, in_=Ls, axis=AX.X, op=OP.max)
                mask0 = gpool.tile([128, E], F32)
                nc.vector.tensor_scalar(
                    out=mask0, in0=Ls, scalar1=m0, scalar2=None, op0=OP.is_equal
                )
                se0 = spool.tile([128, 1], F32)
                scr8 = gpool.tile([128, E], F32)
                nc.vector.tensor_mul(out=scr8, in0=S2, in1=mask0)
                nc.vector.tensor_reduce(out=se0, in_=scr8, axis=AX.X, op=OP.add)
                # mask out the top-1 and find the runner-up
                L1 = gpool.tile([128, E], F32)
                nc.vector.scalar_tensor_tensor(
                    out=L1, in0=mask0, scalar=-1e30, in1=Ls,
                    op0=OP.mult, op1=OP.add,
                )
                m1 = spool.tile([128, 1], F32)
                nc.vector.tensor_reduce(out=m1, in_=L1, axis=AX.X, op=OP.max)
                mask1 = gpool.tile([128, E], F32)
                nc.vector.tensor_scalar(
                    out=mask1, in0=L1, scalar1=m1, scalar2=None, op0=OP.is_equal
                )
                se1 = spool.tile([128, 1], F32)
                scr8b = gpool.tile([128, E], F32)
                nc.vector.tensor_mul(out=scr8b, in0=S2, in1=mask1)
                nc.vector.tensor_reduce(out=se1, in_=scr8b, axis=AX.X, op=OP.add)
                # w0 = sigmoid(m0 - m1), via scale=-1 and bias=m0
                w0 = spool.tile([128, 1], F32)
                nc.scalar.activation(
                    out=w0, in_=m1, func=ACTF.Sigmoid, bias=m0, scale=-1.0
                )
                d01 = spool.tile([128, 1], F32)
                nc.vector.tensor_sub(out=d01, in0=se0, in1=se1)
                # s = w0*(se0-se1) + se1
                nc.vector.tensor_scalar(
                    out=s_cols[:, jn:jn + 1], in0=d01,
                    scalar1=w0, scalar2=se1, op0=OP.mult, op1=OP.add,
                )

            # ---- log-softmax over the full row, write y ----
            nc.gpsimd.dma_start(
                out=s_dram.ap().rearrange("(j p) -> p j", p=128), in_=s_cols
            )
            srow = lpool.tile([1, T], F32)
            nc.gpsimd.dma_start(out=srow, in_=s_dram.ap())
            m1t = lpool.tile([1, 1], F32)
            nc.vector.tensor_reduce(out=m1t, in_=srow, axis=AX.X, op=OP.max)
            m1n = lpool.tile([1, 1], F32)
            nc.vector.tensor_scalar_mul(m1n, m1t, -1.0)
            escr = lpool.tile([1, T], F32)
            z1 = lpool.tile([1, 1], F32)
            nc.scalar.activation(
                out=escr, in_=srow, func=ACTF.Exp, bias=m1n, scale=1.0,
                accum_out=z1,
            )
            lnz = lpool.tile([1, 1], F32)
            nc.scalar.activation(out=lnz, in_=z1, func=ACTF.Ln)
            lse = lpool.tile([1, 1], F32)
            nc.vector.tensor_add(out=lse, in0=m1t, in1=lnz)
            ysb = lpool.tile([1, T], F32)
            nc.vector.tensor_scalar(
                out=ysb, in0=srow, scalar1=lse, scalar2=None, op0=OP.subtract
            )
            nc.gpsimd.dma_start(out=yout.ap(), in_=ysb)

    _legalize_waits(nc)
    return nc


def get_nc(reps: int = 1, variant: str = "full") -> bass.Bass:
    key = f"nc{reps}_{variant}"
    if key not in _CACHE:
        _CACHE[key] = _build_nc(reps, variant)
    return _CACHE[key]


def make_in_maps(x, wg, w1, b1, w2, b2) -> list[dict]:
    x = np.ascontiguousarray(np.asarray(x, dtype=np.float32))
    wg = np.ascontiguousarray(np.asarray(wg, dtype=np.float32))
    w1 = np.ascontiguousarray(np.asarray(w1, dtype=np.float32))
    b1 = np.ascontiguousarray(np.asarray(b1, dtype=np.float32))
    w2 = np.ascontiguousarray(np.asarray(w2, dtype=np.float32))
    b2 = np.ascontiguousarray(np.asarray(b2, dtype=np.float32))
    w1f = np.ascontiguousarray(w1.reshape(E * D, H))
    w2f = np.ascontiguousarray(w2.reshape(E * H, D))
    in_maps = []
    for c in range(NCORES):
        b = c // 2
        in_maps.append({
            "xrow": np.ascontiguousarray(x[b]),
            "wg": wg,
            "w1f": w1f,
            "w2f": w2f,
            "b1f": b1,
            "b2f": b2,
        })
    return in_maps


def _run_once(nc, in_maps) -> np.ndarray:
    res = run_bass_kernel_spmd(nc, in_maps, core_ids=list(range(NCORES)))
    ys = [np.asarray(res.results[c]["y"]).reshape(T) for c in range(NCORES)]
    return np.stack(ys)  # [NCORES, T]


def assemble_output(y_all: np.ndarray) -> np.ndarray:
    """[NCORES*T] or [NCORES, T] per-core rows -> [B, T] (even cores)."""
    y8 = np.asarray(y_all).reshape(NCORES, T)
    return np.ascontiguousarray(y8[0::2]).astype(np.float32)


def _looks_valid(y: np.ndarray) -> bool:
    """Output rows are log-softmax results, so logsumexp(row) must be ~0 and
    everything finite. Catches transient device-state garbage."""
    if not np.all(np.isfinite(y)):
        return False
    m = y.max(axis=1, keepdims=True)
    lse = m + np.log(np.exp(y - m).sum(axis=1, keepdims=True))
    return bool(np.abs(lse).max() < 1e-3)


def kernel(x, wg, w1, b1, w2, b2) -> np.ndarray:
    nc = get_nc()
    in_maps = make_in_maps(x, wg, w1, b1, w2, b2)
    # The axon-relay device occasionally returns one transiently-corrupt
    # execution (stale engine state from a previous tenant). Cores 2b and
    # 2b+1 run identical programs on identical inputs, so their rows must
    # be bit-identical on a clean run -- use that as the integrity check.
    last = None
    for _attempt in range(5):
        y8 = _run_once(nc, in_maps)
        y = assemble_output(y8)
        last = y
        if np.array_equal(y8[0::2], y8[1::2]) and _looks_valid(y):
            return y
    return last
